# revision 14
# baseline (speedup 1.0000x reference)
"""Trainium2 Bass kernel for nn_AttentionHead_6365141532793.

Computes (per batch b):
    q = query @ Wq.T + bq ; k = key @ Wq.T + bq ; v = value @ Wq.T + bq
    out = softmax(q @ k.T / sqrt(D)) @ v

Sharding: 8 cores = 4 batches x 2 query-slabs (2048 rows each). Each core
holds the full key/value of its batch, so softmax rows are complete per
core and no collectives are needed.

Layout note: rows are DMA'd in pairs (2KB contiguous HBM lines, 2x the
descriptor efficiency of single 1KB rows), so a 128-row "chunk" holds
rows {256c + 2p + j} for j = chunk parity. The permutation is consistent
across q/k/v and undone by the paired output-write access pattern.

Per-core algorithm (main-loop matmuls in fp8e4 with DoubleRow perf mode:
2 elem/cycle, 256-deep contraction folded into one matmul):
  - load Wq / q-slab / key as f32, cast bf16 (DVE), transpose 128x128
    blocks on the TensorEngine in bf16; value cast straight to fp8 (DVE)
  - q_projT[o, m] = wqT.T @ qT; k_projT[o, n] = wqT.T @ kT -- bias add
    fused into the PSUM->SBUF fp8 cast on the ScalarEngine
  - for each m-super (512 query rows), for each pair of key chunks:
      scoresT[n, m] = k_projT.T @ q_projT     (1 DoubleRow mm per chunk)
      pT = exp(scoresT / 16) in fp8           (ScalarE for most groups;
           2/16 groups use a Schraudolph bit-trick on the DVE: fp8 bits
           = round(scores * 8*log2e/16 + (56 - 8*0.0597)) as int8)
      UT[d, m]   += v_pair.T @ pT             (DoubleRow, PSUM accum)
      s[m]       += rowsum(pT)  (DVE adds for g<11, ones-matmul for the
           last 5 groups -- balances PE vs DVE occupancy)
    The score matmul for group g+1 is issued before exp/UT of group g so
    the in-order PE queue never stalls on the ScalarEngine.
  - epilogue per m-super: recip = reciprocal_approx_fast(s_dve + s_pe);
    per m-chunk: normalize UT while casting bf16, project, + bq, DMA out.
No max-subtraction in softmax: scores/16 are O(1) for this distribution,
exp is safe and softmax is shift-invariant anyway.
"""

import contextlib
import dataclasses
import itertools

import numpy as np

B, S, D = 4, 4096, 256
NCORES = 8
NQ = S // 2          # query rows per core
P = 128
DC = D // P          # 2 chunks of the d/o dimension
MSUP = 512           # m-super: moving free dim per score matmul
NSUP = NQ // MSUP    # 4 m-supers per core
NKC = S // P         # 32 key chunks
GN = 2               # key chunks per group (DoubleRow pair)
NG = NKC // GN       # 16 groups
SCALE = 1.0 / 16.0   # 1/sqrt(D)
NPE_SUM = 16         # trailing groups per m-super whose rowsum runs on PE
DVE_EXP = (3, 11)    # groups per m-super whose exp runs on the DVE
# Schraudolph fp8e4 exp bits: round(x * 8*log2e/16 + 8*(7 - 0.0597))
EXP_A = 8 * 1.4426950408889634 / 16
EXP_B = 56.0 - 8 * 0.0597

_CACHE = {}


def _build(num_devices=NCORES):
    import concourse.mybir as mybir
    import concourse.tile as tile
    from concourse import bacc
    from concourse.masks import make_identity

    f32 = mybir.dt.float32
    bf16 = mybir.dt.bfloat16
    fp8 = mybir.dt.float8e4
    i8 = mybir.dt.int8
    FT = mybir.ActivationFunctionType
    ALU = mybir.AluOpType
    DR = mybir.MatmulPerfMode.DoubleRow

    nc = bacc.Bacc("TRN2", target_bir_lowering=False, debug=False,
                   num_devices=num_devices)
    q_ext = nc.dram_tensor("query", [NQ, D], f32, kind="ExternalInput").ap()
    k_ext = nc.dram_tensor("key", [S, D], f32, kind="ExternalInput").ap()
    v_ext = nc.dram_tensor("value", [S, D], f32, kind="ExternalInput").ap()
    w_ext = nc.dram_tensor("Wq", [D, D], f32, kind="ExternalInput").ap()
    b_ext = nc.dram_tensor("bq", [D], f32, kind="ExternalInput").ap()
    o_ext = nc.dram_tensor("out", [NQ, D], f32, kind="ExternalOutput").ap()

    with contextlib.ExitStack() as ctx:
        tc = ctx.enter_context(tile.TileContext(nc))
        singles = ctx.enter_context(tc.tile_pool(name="singles", bufs=1))
        fstage = ctx.enter_context(tc.tile_pool(name="fstage", bufs=3))
        bstage = ctx.enter_context(tc.tile_pool(name="bstage", bufs=3))
        ptpool = ctx.enter_context(tc.tile_pool(name="ptpool", bufs=3))
        utsbp = ctx.enter_context(tc.tile_pool(name="utsbp", bufs=2))
        rsbp = ctx.enter_context(tc.tile_pool(name="rsbp", bufs=2))
        osbp = ctx.enter_context(tc.tile_pool(name="osbp", bufs=3))
        # PSUM budget (16KB/partition): st 2x4KB + ut 4KB + s 2KB + o 2KB
        ps_st = ctx.enter_context(tc.tile_pool(name="ps_st", bufs=2, space="PSUM"))
        ps_ut = ctx.enter_context(tc.tile_pool(name="ps_ut", bufs=1, space="PSUM"))
        ps_s = ctx.enter_context(tc.tile_pool(name="ps_s", bufs=1, space="PSUM"))
        ps_o = ctx.enter_context(tc.tile_pool(name="ps_o", bufs=1, space="PSUM"))

        # ---------------- constants ----------------
        ident_f = singles.tile([P, P], f32, tag="identf")
        make_identity(nc, ident_f)
        ident = singles.tile([P, P], bf16, tag="ident")
        nc.vector.tensor_copy(ident, ident_f)
        # ones: rowsum matmul -> s replicated on all partitions
        ones_f8 = singles.tile([P, GN, P], fp8, tag="ones")
        nc.vector.memset(ones_f8, 1.0)
        # bq_pc[p, oc] = bq[oc*P + p] (per-partition bias for projT layouts)
        bq_pc = singles.tile([P, DC], f32, tag="bq_pc")
        nc.sync.dma_start(out=bq_pc, in_=b_ext.rearrange("(c p) -> p c", p=P))
        # bq_full[p, o] = bq[o] -- broadcast across partitions (DMA step-0)
        bq_full = singles.tile([P, D], f32, tag="bq_full")
        b_bc = dataclasses.replace(
            b_ext, ap=[[0, P]] + [list(c) for c in b_ext.ap])
        nc.sync.dma_start(out=bq_full, in_=b_bc)

        # prologue PSUM: transposes from ps_st/ps_o, projections ps_ut/ps_s
        # (tags must match each pool's main-loop tag so regions are shared)
        tp_pools = itertools.cycle([(ps_st, "st"), (ps_o, "o")])
        pp_pools = itertools.cycle([(ps_ut, "ut"), (ps_s, "s")])

        # wqT[p, dc, o] = Wq[o, dc*P + p]  (bf16 PE transpose of bf16 cast)
        wq_f = fstage.tile([P, DC, D], f32, tag="fst", name="wq_f")
        nc.sync.dma_start(out=wq_f, in_=w_ext.rearrange("(c p) d -> p c d", p=P))
        wq_b = bstage.tile([P, DC, D], bf16, tag="bst", name="wq_b")
        nc.vector.tensor_copy(wq_b, wq_f)
        wqT = singles.tile([P, DC, D], bf16, tag="wqT")
        for dcc in range(DC):
            pool, tag = next(tp_pools)
            tp = pool.tile([P, DC, P], bf16, tag=tag, name=f"wqt_{dcc}")
            for oc in range(DC):
                nc.tensor.transpose(tp[:, oc, :],
                                    wq_b[:, oc, dcc * P:(dcc + 1) * P], ident)
            nc.vector.tensor_copy(wqT[:, dcc, :], tp)

        # ------------- load + cast + transpose activations -------------
        # paired-row load: tile[p, gi, j, d] = src[(c*4+gi)*256 + 2p + j, d]
        def load_paired(ext, c, nm):
            nat = ext.rearrange("(c p j) d -> p c (j d)", p=P, j=2)
            tf = fstage.tile([P, 4, 2, D], f32, tag="fst", name=f"f{nm}_{c}")
            nc.sync.dma_start(
                out=tf.rearrange("p a b d -> p a (b d)"),
                in_=nat[:, c * 4:(c + 1) * 4, :])
            return tf

        def transpose_tile(tf, c, dst, nm):
            # cast bf16 then PE-transpose; chunk index mc = 8c + 2gi + j
            tb = bstage.tile([P, 4, 2, D], bf16, tag="bst", name=f"b{nm}_{c}")
            nc.vector.tensor_copy(tb, tf)
            for dcc in range(DC):
                for h in range(2):
                    pool, tag = next(tp_pools)
                    tp = pool.tile([P, 4, P], bf16, tag=tag,
                                   name=f"t{nm}_{c}_{dcc}_{h}")
                    for i4 in range(4):
                        gi, j = 2 * h + i4 // 2, i4 % 2
                        nc.tensor.transpose(
                            tp[:, i4, :], tb[:, gi, j, dcc * P:(dcc + 1) * P],
                            ident)
                    mc0 = 8 * c + 4 * h
                    nc.vector.tensor_copy(dst[:, dcc, mc0 * P:(mc0 + 4) * P],
                                          tp)

        def project(srcT, dst, nsi):
            # dst[p, oc, nsi-slab] = (Wq srcT + bq) in fp8; bias-add on ACT
            nsl = slice(nsi * MSUP, (nsi + 1) * MSUP)
            for oc in range(DC):
                pool, tag = next(pp_pools)
                pp = pool.tile([P, MSUP], f32, tag=tag,
                               name=f"pp_{id(dst)}_{oc}_{nsi}")
                for dcc in range(DC):
                    nc.tensor.matmul(pp, lhsT=wqT[:, dcc, oc * P:(oc + 1) * P],
                                     rhs=srcT[:, dcc, nsl],
                                     start=(dcc == 0), stop=(dcc == DC - 1))
                nc.scalar.activation(out=dst[:, oc, nsl], in_=pp,
                                     func=FT.Identity,
                                     bias=bq_pc[:, oc:oc + 1])

        # q: 2 tiles of 1024 rows
        qT = singles.tile([P, DC, NQ], bf16, tag="qT")
        q_pT = singles.tile([P, DC, NQ], fp8, tag="q_pT")
        for c in range(NQ // (8 * P)):
            tf = load_paired(q_ext, c, "q")
            transpose_tile(tf, c, qT, "q")
            for nsi in (2 * c, 2 * c + 1):
                project(qT, q_pT, nsi)

        # k/v interleaved per 1024-row tile so v arrives before main loop
        kT = singles.tile([P, DC, S], bf16, tag="kT")
        k_pT = singles.tile([P, DC, S], fp8, tag="k_pT")
        v_f8 = singles.tile([P, NKC, D], fp8, tag="v_f8")
        for c in range(S // (8 * P)):
            tf = load_paired(k_ext, c, "k")
            vf = load_paired(v_ext, c, "v")
            transpose_tile(tf, c, kT, "k")
            for nsi in (2 * c, 2 * c + 1):
                project(kT, k_pT, nsi)
            nc.vector.tensor_copy(v_f8[:, c * 8:(c + 1) * 8, :],
                                  vf.rearrange("p a b d -> p (a b) d"))

        # ------------- attention main loop (sw-pipelined) -------------
        uts, sps, sas, sts = {}, {}, {}, {}
        o_paired = o_ext.rearrange("(c p j) d -> p c j d", p=P, j=2)

        def epilogue(msi):
            ut_ps = uts.pop(msi)
            s_ps = sps.pop(msi)
            s_ac = sas.pop(msi)
            recip = rsbp.tile([P, MSUP], f32, tag="recip", name=f"r_{msi}")
            if NPE_SUM >= NG:
                nc.vector.reciprocal_approx_fast(recip, s_ps)
            else:
                stot = rsbp.tile([P, MSUP], f32, tag="stot",
                                 name=f"stot_{msi}")
                nc.vector.tensor_add(stot, s_ac, s_ps)
                nc.vector.reciprocal_approx_fast(recip, stot)
            ut_sb = utsbp.tile([P, DC, MSUP], bf16, tag="utsb",
                               name=f"utsb_{msi}")
            for mc in range(MSUP // P):
                msl = slice(mc * P, (mc + 1) * P)
                for dcc in range(DC):
                    nc.vector.tensor_mul(ut_sb[:, dcc, msl],
                                         ut_ps[:, dcc, msl], recip[:, msl])
                o_ps = ps_o.tile([P, D], f32, tag="o", name=f"o_{msi}_{mc}")
                for dcc in range(DC):
                    nc.tensor.matmul(o_ps,
                                     lhsT=ut_sb[:, dcc, msl],
                                     rhs=wqT[:, dcc, :],
                                     start=(dcc == 0), stop=(dcc == DC - 1),
                                     skip_group_check=True)
                o_sb = osbp.tile([P, D], f32, tag="osb", name=f"ob_{msi}_{mc}")
                nc.vector.tensor_add(o_sb, o_ps, bq_full)
                mcg = msi * (MSUP // P) + mc
                nc.sync.dma_start(out=o_paired[:, mcg // 2, mcg % 2, :],
                                  in_=o_sb)

        TT = NSUP * NG
        for t in range(TT + 1):
            if t < TT:
                # issue score matmuls for group t (one group of lookahead)
                msi, g = divmod(t, NG)
                if g == 0:
                    uts[msi] = ps_ut.tile([P, DC, MSUP], f32, tag="ut",
                                          name=f"ut_{msi}")
                    sps[msi] = ps_s.tile([P, MSUP], f32, tag="s",
                                         name=f"s_{msi}")
                    sas[msi] = rsbp.tile([P, MSUP], f32, tag="sacc",
                                         name=f"sa_{msi}")
                st = ps_st.tile([P, GN, MSUP], f32, tag="st", name=f"st_{t}")
                msl = slice(msi * MSUP, (msi + 1) * MSUP)
                for i in range(GN):
                    kc = g * GN + i
                    nc.tensor.matmul(st[:, i, :],
                                     lhsT=k_pT[:, :, kc * P:(kc + 1) * P],
                                     rhs=q_pT[:, :, msl],
                                     start=True, stop=True, perf_mode=DR,
                                     skip_group_check=True)
                sts[t] = st
            if t > 0:
                tp_ = t - 1
                msi, g = divmod(tp_, NG)
                st = sts.pop(tp_)
                pt = ptpool.tile([P, GN, MSUP], fp8, tag="pt", name=f"pt_{tp_}")
                if g in DVE_EXP:
                    nc.vector.tensor_scalar(pt.bitcast(i8), st, EXP_A, EXP_B,
                                            op0=ALU.mult, op1=ALU.add)
                else:
                    nc.scalar.activation(out=pt, in_=st, func=FT.Exp,
                                         scale=SCALE)
                for dcc in range(DC):
                    nc.tensor.matmul(uts[msi][:, dcc, :],
                                     lhsT=v_f8[:, GN * g:GN * (g + 1),
                                               dcc * P:(dcc + 1) * P],
                                     rhs=pt,
                                     start=(g == 0), stop=(g == NG - 1),
                                     perf_mode=DR, skip_group_check=True)
                if g >= NG - NPE_SUM:
                    nc.tensor.matmul(sps[msi], lhsT=ones_f8, rhs=pt,
                                     start=(g == NG - NPE_SUM),
                                     stop=(g == NG - 1),
                                     perf_mode=DR, skip_group_check=True)
                elif g == 0:
                    nc.vector.tensor_add(sas[msi], pt[:, 0, :], pt[:, 1, :])
                else:
                    for i in range(GN):
                        nc.vector.tensor_add(sas[msi], sas[msi], pt[:, i, :])
                if g == NG - 1:
                    epilogue(msi)

    nc.finalize()
    return nc


def _get_nc():
    if "nc" not in _CACHE:
        _CACHE["nc"] = _build()
    return _CACHE["nc"]


def kernel(query, key, value, Wq, bq):
    from concourse.bass_utils import run_bass_kernel_spmd

    nc = _get_nc()
    in_maps = []
    for core in range(NCORES):
        b, h = core // 2, core % 2
        in_maps.append({
            "query": np.ascontiguousarray(query[b, h * NQ:(h + 1) * NQ, :],
                                          dtype=np.float32),
            "key": np.ascontiguousarray(key[b], dtype=np.float32),
            "value": np.ascontiguousarray(value[b], dtype=np.float32),
            "Wq": np.ascontiguousarray(Wq, dtype=np.float32),
            "bq": np.ascontiguousarray(bq, dtype=np.float32),
        })
    res = run_bass_kernel_spmd(nc, in_maps, core_ids=list(range(NCORES)))
    out = np.empty((B, S, D), np.float32)
    for core in range(NCORES):
        b, h = core // 2, core % 2
        out[b, h * NQ:(h + 1) * NQ, :] = res.results[core]["out"]
    return out


# revision 24
# speedup vs baseline: 1.1103x; 1.1103x over previous
"""Trainium2 Bass kernel for nn_AttentionHead_6365141532793.

Computes (per batch b):
    q = query @ Wq.T + bq ; k = key @ Wq.T + bq ; v = value @ Wq.T + bq
    out = softmax(q @ k.T / sqrt(D)) @ v

Sharding: 8 cores = 4 batches x 2 query-slabs (2048 rows each). Each core
holds the full key/value of its batch, so softmax rows are complete per
core and no collectives are needed.

Layout note: rows are DMA'd in pairs (2KB contiguous HBM lines, 2x the
descriptor efficiency of single 1KB rows), so a 128-row "chunk" holds
rows {256c + 2p + j} for j = chunk parity. The permutation is consistent
across q/k/v and undone by the paired output-write access pattern.

Main-loop matmuls run in fp8e4 with DoubleRow perf mode (2 elem/cycle,
256-deep contraction folded into one matmul). The q-projection is
pre-scaled by A = 8*log2e/16 so scores arrive in "fp8 exponent bit"
units: the ScalarE exp uses scale 1/(8*log2e), and a few groups per
m-super compute exp on the DVE as a single Schraudolph add:
fp8e4 bits = rne(A*score + 8*(7-sigma)) written as int8.

Emission interleaves the prologue with the main loop: after each k/v
tile (1024 keys) lands, its transposes + projections and the next 4
score/exp/PV groups of m-super 0 are issued, so compute starts as soon
as the first tile arrives instead of after all loads. The score matmul
for group g+1 is always issued before exp/UT of group g so the in-order
PE queue never stalls on the exp engines.

Rowsum of exp(scores): ones-matmul on the PE for the last NPE_SUM groups
per m-super, ping-pong DVE adds for the rest (balances PE vs DVE).
Epilogue per m-super: recip = reciprocal_approx_fast(s_dve + s_pe); per
m-chunk: normalize UT while casting bf16, project with wqT, add bq, DMA
out. (Projection of U = P@V applied after attention is exact:
P (V Wq^T + 1 bq^T) = (P V) Wq^T + s bq^T.)
No max-subtraction in softmax: scores/16 are O(1) for this distribution,
exp is safe and softmax is shift-invariant anyway.
"""

import contextlib
import dataclasses
import itertools

import numpy as np

B, S, D = 4, 4096, 256
NCORES = 8
NQ = S // 2          # query rows per core
P = 128
DC = D // P          # 2 chunks of the d/o dimension
MSUP = 512           # m-super: moving free dim per score matmul
NSUP = NQ // MSUP    # 4 m-supers per core
NKC = S // P         # 32 key chunks
GN = 2               # key chunks per group (DoubleRow pair)
NG = NKC // GN       # 16 groups per m-super
NTIL = S // (8 * P)  # 4 k/v tiles (1024 rows each)
SCALE = 1.0 / 16.0   # 1/sqrt(D)
NPE_SUM = 5          # trailing groups per m-super whose rowsum runs on PE
DVE_EXP = ()         # groups per m-super whose exp runs on the DVE
# Scores are pre-scaled by A (folded into the q projection); exp via
# Schraudolph bits rne(A*score + B) on DVE, or Exp(score*A/(8 log2e)) on ACT
EXP_A = 8 * 1.4426950408889634 / 16
EXP_B = 56.0 - 8 * 0.0597
ACT_SCALE = 1.0 / (8 * 1.4426950408889634)

_CACHE = {}
DEBUG_DUMP = False


def _build(num_devices=NCORES):
    import concourse.mybir as mybir
    import concourse.tile as tile
    from concourse import bacc
    from concourse.masks import make_identity

    f32 = mybir.dt.float32
    bf16 = mybir.dt.bfloat16
    fp8 = mybir.dt.float8e4
    i8 = mybir.dt.int8
    FT = mybir.ActivationFunctionType
    ALU = mybir.AluOpType
    DR = mybir.MatmulPerfMode.DoubleRow

    nc = bacc.Bacc("TRN2", target_bir_lowering=False, debug=False,
                   num_devices=num_devices)
    q_ext = nc.dram_tensor("query", [NQ, D], f32, kind="ExternalInput").ap()
    k_ext = nc.dram_tensor("key", [S, D], f32, kind="ExternalInput").ap()
    v_ext = nc.dram_tensor("value", [S, D], f32, kind="ExternalInput").ap()
    w_ext = nc.dram_tensor("Wq", [D, D], f32, kind="ExternalInput").ap()
    b_ext = nc.dram_tensor("bq", [D], f32, kind="ExternalInput").ap()
    o_ext = nc.dram_tensor("out", [NQ, D], f32, kind="ExternalOutput").ap()
    if DEBUG_DUMP:
        dbg_sa = nc.dram_tensor("dbg_sa", [NSUP, P, MSUP], f32,
                                kind="ExternalOutput").ap()
        dbg_sp = nc.dram_tensor("dbg_sp", [NSUP, P, MSUP], f32,
                                kind="ExternalOutput").ap()
        dbg_pt = nc.dram_tensor("dbg_pt", [P, GN, MSUP], f32,
                                kind="ExternalOutput").ap()

    with contextlib.ExitStack() as ctx:
        tc = ctx.enter_context(tile.TileContext(nc))
        singles = ctx.enter_context(tc.tile_pool(name="singles", bufs=1))
        fstage = ctx.enter_context(tc.tile_pool(name="fstage", bufs=3))
        bstage = ctx.enter_context(tc.tile_pool(name="bstage", bufs=3))
        ptpool = ctx.enter_context(tc.tile_pool(name="ptpool", bufs=3))
        utsbp = ctx.enter_context(tc.tile_pool(name="utsbp", bufs=2))
        rsbp = ctx.enter_context(tc.tile_pool(name="rsbp", bufs=2))
        osbp = ctx.enter_context(tc.tile_pool(name="osbp", bufs=3))
        # PSUM budget (16KB/partition): st 2x4KB + ut 4KB + s 2KB + o 2KB.
        # Prologue transposes (tp) and projections (pp) borrow st/o slots so
        # they can interleave with main-loop tiles without deadlock.
        ps_st = ctx.enter_context(tc.tile_pool(name="ps_st", bufs=2, space="PSUM"))
        ps_ut = ctx.enter_context(tc.tile_pool(name="ps_ut", bufs=1, space="PSUM"))
        ps_s = ctx.enter_context(tc.tile_pool(name="ps_s", bufs=1, space="PSUM"))
        ps_o = ctx.enter_context(tc.tile_pool(name="ps_o", bufs=1, space="PSUM"))

        # ---------------- constants ----------------
        ident_f = singles.tile([P, P], f32, tag="identf")
        make_identity(nc, ident_f)
        ident = singles.tile([P, P], bf16, tag="ident")
        nc.vector.tensor_copy(ident, ident_f)
        # ones: rowsum matmul -> s replicated on all partitions
        ones_f8 = singles.tile([P, GN, P], fp8, tag="ones")
        nc.vector.memset(ones_f8, 1.0)
        ones_bf = singles.tile([P, P], bf16, tag="ones_bf")
        nc.vector.memset(ones_bf, 1.0)
        # bq_pc[p, oc] = bq[oc*P + p] (per-partition bias for projT layouts)
        bq_pc = singles.tile([P, DC], f32, tag="bq_pc")
        nc.sync.dma_start(out=bq_pc, in_=b_ext.rearrange("(c p) -> p c", p=P))
        bq_pcA = singles.tile([P, DC], f32, tag="bq_pcA")
        nc.vector.tensor_scalar_mul(bq_pcA, bq_pc, EXP_A)
        # bq_full[p, o] = bq[o] -- broadcast across partitions (DMA step-0)
        bq_full = singles.tile([P, D], f32, tag="bq_full")
        b_bc = dataclasses.replace(
            b_ext, ap=[[0, P]] + [list(c) for c in b_ext.ap])
        nc.sync.dma_start(out=bq_full, in_=b_bc)

        tp_pools = itertools.cycle([(ps_st, "st"), (ps_o, "o")])
        pp_pools = itertools.cycle([(ps_st, "st"), (ps_o, "o")])

        # wqT[p, dc, o] = Wq[o, dc*P + p]  (bf16 PE transpose of bf16 cast)
        wq_f = fstage.tile([P, DC, D], f32, tag="fst", name="wq_f")
        nc.sync.dma_start(out=wq_f, in_=w_ext.rearrange("(c p) d -> p c d", p=P))
        wq_b = bstage.tile([P, DC, D], bf16, tag="bst", name="wq_b")
        nc.vector.tensor_copy(wq_b, wq_f)
        wqT = singles.tile([P, DC, D], bf16, tag="wqT")
        for dcc in range(DC):
            pool, tag = next(tp_pools)
            tp = pool.tile([P, DC, P], bf16, tag=tag, name=f"wqt_{dcc}")
            for oc in range(DC):
                nc.tensor.transpose(tp[:, oc, :],
                                    wq_b[:, oc, dcc * P:(dcc + 1) * P], ident)
            nc.vector.tensor_copy(wqT[:, dcc, :], tp)

        # ------------- load + cast + transpose helpers -------------
        # paired-row load: tile[p, gi, j, d] = src[(c*4+gi)*256 + 2p + j, d]
        def load_paired(ext, c, nm):
            nat = ext.rearrange("(c p j) d -> p c (j d)", p=P, j=2)
            tf = fstage.tile([P, 4, 2, D], f32, tag="fst", name=f"f{nm}_{c}")
            nc.sync.dma_start(
                out=tf.rearrange("p a b d -> p a (b d)"),
                in_=nat[:, c * 4:(c + 1) * 4, :])
            return tf

        def transpose_tile(tf, c, dst, nm):
            # cast bf16 then PE-transpose; chunk index mc = 8c + 2gi + j
            tb = bstage.tile([P, 4, 2, D], bf16, tag="bst", name=f"b{nm}_{c}")
            nc.vector.tensor_copy(tb, tf)
            for dcc in range(DC):
                for h in range(2):
                    pool, tag = next(tp_pools)
                    tp = pool.tile([P, 4, P], bf16, tag=tag,
                                   name=f"t{nm}_{c}_{dcc}_{h}")
                    for i4 in range(4):
                        gi, j = 2 * h + i4 // 2, i4 % 2
                        nc.tensor.transpose(
                            tp[:, i4, :], tb[:, gi, j, dcc * P:(dcc + 1) * P],
                            ident)
                    mc0 = 8 * c + 4 * h
                    nc.vector.tensor_copy(dst[:, dcc, mc0 * P:(mc0 + 4) * P],
                                          tp)

        def project(srcT, dst, nsi, scaled):
            # dst[p, oc, nsi-slab] = (Wq srcT + bq) [* A] in fp8; add on ACT
            nsl = slice(nsi * MSUP, (nsi + 1) * MSUP)
            for oc in range(DC):
                pool, tag = next(pp_pools)
                pp = pool.tile([P, MSUP], f32, tag=tag,
                               name=f"pp_{id(dst)}_{oc}_{nsi}")
                for dcc in range(DC):
                    nc.tensor.matmul(pp, lhsT=wqT[:, dcc, oc * P:(oc + 1) * P],
                                     rhs=srcT[:, dcc, nsl],
                                     start=(dcc == 0), stop=(dcc == DC - 1))
                if scaled:
                    nc.scalar.activation(out=dst[:, oc, nsl], in_=pp,
                                         func=FT.Identity, scale=EXP_A,
                                         bias=bq_pcA[:, oc:oc + 1])
                else:
                    nc.scalar.activation(out=dst[:, oc, nsl], in_=pp,
                                         func=FT.Identity,
                                         bias=bq_pc[:, oc:oc + 1])

        qT = singles.tile([P, DC, NQ], bf16, tag="qT")
        q_pT = singles.tile([P, DC, NQ], fp8, tag="q_pT")
        kT = singles.tile([P, DC, S], bf16, tag="kT")
        k_pT = singles.tile([P, DC, S], fp8, tag="k_pT")
        v_f8 = singles.tile([P, NKC, D], fp8, tag="v_f8")

        # ------------- main-loop emission helpers -------------
        uts, sps, saps, sts = {}, {}, {}, {}
        o_paired = o_ext.rearrange("(c p j) d -> p c j d", p=P, j=2)

        def epilogue(msi):
            ut_ps = uts.pop(msi)
            s_ps = sps.pop(msi)
            recip = rsbp.tile([P, MSUP], f32, tag="recip", name=f"r_{msi}")
            if NPE_SUM < NG:
                # fold the DVE per-partition partial sums into s_ps: one
                # bf16 cast + a 128-contraction ones-matmul per chunk pair
                sa, _ = saps.pop(msi)
                sab = rsbp.tile([P, GN, MSUP], bf16, tag="sab",
                                name=f"sab_{msi}")
                nc.vector.tensor_copy(sab, sa)
                for i in range(GN):
                    nc.tensor.matmul(s_ps, lhsT=ones_bf, rhs=sab[:, i, :],
                                     start=False, stop=(i == GN - 1),
                                     skip_group_check=True)
            nc.vector.reciprocal_approx_fast(recip, s_ps)
            ut_sb = utsbp.tile([P, DC, MSUP], bf16, tag="utsb",
                               name=f"utsb_{msi}")
            for mc in range(MSUP // P):
                msl = slice(mc * P, (mc + 1) * P)
                for dcc in range(DC):
                    nc.vector.tensor_mul(ut_sb[:, dcc, msl],
                                         ut_ps[:, dcc, msl], recip[:, msl])
                o_ps = ps_o.tile([P, D], f32, tag="o", name=f"o_{msi}_{mc}")
                for dcc in range(DC):
                    nc.tensor.matmul(o_ps,
                                     lhsT=ut_sb[:, dcc, msl],
                                     rhs=wqT[:, dcc, :],
                                     start=(dcc == 0), stop=(dcc == DC - 1),
                                     skip_group_check=True)
                o_sb = osbp.tile([P, D], f32, tag="osb", name=f"ob_{msi}_{mc}")
                nc.vector.tensor_add(o_sb, o_ps, bq_full)
                mcg = msi * (MSUP // P) + mc
                nc.sync.dma_start(out=o_paired[:, mcg // 2, mcg % 2, :],
                                  in_=o_sb)

        TT = NSUP * NG

        pts = {}

        def emit_score(t):
            msi, g = divmod(t, NG)
            if g == 0:
                uts[msi] = ps_ut.tile([P, DC, MSUP], f32, tag="ut",
                                      name=f"ut_{msi}")
                sps[msi] = ps_s.tile([P, MSUP], f32, tag="s", name=f"s_{msi}")
                if NPE_SUM < NG:
                    saps[msi] = [
                        rsbp.tile([P, GN, MSUP], f32, tag="sacca",
                                  name=f"sa_{msi}"),
                        rsbp.tile([P, GN, MSUP], f32, tag="saccb",
                                  name=f"sb_{msi}"),
                    ]
            st = ps_st.tile([P, GN, MSUP], f32, tag="st", name=f"st_{t}")
            msl = slice(msi * MSUP, (msi + 1) * MSUP)
            for i in range(GN):
                kc = g * GN + i
                nc.tensor.matmul(st[:, i, :],
                                 lhsT=k_pT[:, :, kc * P:(kc + 1) * P],
                                 rhs=q_pT[:, :, msl],
                                 start=True, stop=True, perf_mode=DR,
                                 skip_group_check=True)
            # exp issued immediately so ACT/DVE get a full group of lead
            pt = ptpool.tile([P, GN, MSUP], fp8, tag="pt", name=f"pt_{t}")
            if g in DVE_EXP:
                nc.vector.tensor_scalar_add(pt.bitcast(i8), st, EXP_B)
            else:
                nc.scalar.activation(out=pt, in_=st, func=FT.Exp,
                                     scale=ACT_SCALE)
            sts[t] = st
            pts[t] = pt

        def consume(t):
            msi, g = divmod(t, NG)
            sts.pop(t)
            pt = pts.pop(t)
            for dcc in range(DC):
                nc.tensor.matmul(uts[msi][:, dcc, :],
                                 lhsT=v_f8[:, GN * g:GN * (g + 1),
                                           dcc * P:(dcc + 1) * P],
                                 rhs=pt,
                                 start=(g == 0), stop=(g == NG - 1),
                                 perf_mode=DR, skip_group_check=True)
            if g >= NG - NPE_SUM:
                # per-group partition reduction on the PE (replicated rows)
                nc.tensor.matmul(sps[msi], lhsT=ones_f8, rhs=pt,
                                 start=(g == NG - NPE_SUM),
                                 stop=(NPE_SUM >= NG and g == NG - 1),
                                 perf_mode=DR, skip_group_check=True)
            elif g == 0:
                nc.vector.tensor_copy(saps[msi][0], pt)
            else:
                # ping-pong accumulate per-partition partial sums on DVE;
                # the epilogue folds the partition reduction into s_ps
                sa, sb = saps[msi]
                nc.vector.tensor_add(sb, sa, pt)
                saps[msi] = [sb, sa]
            if g == NG - 1:
                epilogue(msi)

        tcur = [0]

        def advance(upto):
            while tcur[0] < upto:
                t = tcur[0]
                if t < TT:
                    emit_score(t)
                if t > 0:
                    consume(t - 1)
                tcur[0] += 1

        # ------------- interleaved prologue + main loop -------------
        # q tile 0 -> q_pT slabs for m-supers 0,1
        tf = load_paired(q_ext, 0, "q")
        transpose_tile(tf, 0, qT, "q")
        for nsi in (0, 1):
            project(qT, q_pT, nsi, scaled=True)

        for c in range(NTIL):
            tf = load_paired(k_ext, c, "k")
            vf = load_paired(v_ext, c, "v")
            transpose_tile(tf, c, kT, "k")
            for nsi in (2 * c, 2 * c + 1):
                project(kT, k_pT, nsi, scaled=False)
            nc.vector.tensor_copy(v_f8[:, c * 8:(c + 1) * 8, :],
                                  vf.rearrange("p a b d -> p (a b) d"))
            advance(4 * (c + 1))
            if c == 0:
                # q tile 1 -> slabs for m-supers 2,3 (needed from t=32)
                tf = load_paired(q_ext, 1, "q")
                transpose_tile(tf, 1, qT, "q")
                for nsi in (2, 3):
                    project(qT, q_pT, nsi, scaled=True)

        advance(TT + 1)

    nc.finalize()
    return nc


def _get_nc():
    if "nc" not in _CACHE:
        _CACHE["nc"] = _build()
    return _CACHE["nc"]


def kernel(query, key, value, Wq, bq):
    from concourse.bass_utils import run_bass_kernel_spmd

    nc = _get_nc()
    in_maps = []
    for core in range(NCORES):
        b, h = core // 2, core % 2
        in_maps.append({
            "query": np.ascontiguousarray(query[b, h * NQ:(h + 1) * NQ, :],
                                          dtype=np.float32),
            "key": np.ascontiguousarray(key[b], dtype=np.float32),
            "value": np.ascontiguousarray(value[b], dtype=np.float32),
            "Wq": np.ascontiguousarray(Wq, dtype=np.float32),
            "bq": np.ascontiguousarray(bq, dtype=np.float32),
        })
    res = run_bass_kernel_spmd(nc, in_maps, core_ids=list(range(NCORES)))
    out = np.empty((B, S, D), np.float32)
    for core in range(NCORES):
        b, h = core // 2, core % 2
        out[b, h * NQ:(h + 1) * NQ, :] = res.results[core]["out"]
    return out


# revision 25
# speedup vs baseline: 1.1609x; 1.0455x over previous
"""Trainium2 Bass kernel for nn_AttentionHead_6365141532793.

Computes (per batch b):
    q = query @ Wq.T + bq ; k = key @ Wq.T + bq ; v = value @ Wq.T + bq
    out = softmax(q @ k.T / sqrt(D)) @ v

Sharding: 8 cores = 4 batches x 2 query-slabs (2048 rows each). Each core
holds the full key/value of its batch, so softmax rows are complete per
core and no collectives are needed.

Layout note: rows are DMA'd in pairs (2KB contiguous HBM lines, 2x the
descriptor efficiency of single 1KB rows), so a 128-row "chunk" holds
rows {256c + 2p + j} for j = chunk parity. The permutation is consistent
across q/k/v and undone by the paired output-write access pattern.

Main-loop matmuls run in fp8e4 with DoubleRow perf mode (2 elem/cycle,
256-deep contraction folded into one matmul). The q-projection is
pre-scaled by A = 8*log2e/16 so scores arrive in "fp8 exponent bit"
units: the ScalarE exp uses scale 1/(8*log2e), and a few groups per
m-super compute exp on the DVE as a single Schraudolph add:
fp8e4 bits = rne(A*score + 8*(7-sigma)) written as int8.

Emission interleaves the prologue with the main loop: after each k/v
tile (1024 keys) lands, its transposes + projections and the next 4
score/exp/PV groups of m-super 0 are issued, so compute starts as soon
as the first tile arrives instead of after all loads. The score matmul
for group g+1 is always issued before exp/UT of group g so the in-order
PE queue never stalls on the exp engines.

Rowsum of exp(scores): ones-matmul on the PE for the last NPE_SUM groups
per m-super, ping-pong DVE adds for the rest (balances PE vs DVE).
Epilogue per m-super: recip = reciprocal_approx_fast(s_dve + s_pe); per
m-chunk: normalize UT while casting bf16, project with wqT, add bq, DMA
out. (Projection of U = P@V applied after attention is exact:
P (V Wq^T + 1 bq^T) = (P V) Wq^T + s bq^T.)
No max-subtraction in softmax: scores/16 are O(1) for this distribution,
exp is safe and softmax is shift-invariant anyway.
"""

import contextlib
import dataclasses
import itertools

import numpy as np

B, S, D = 4, 4096, 256
NCORES = 8
NQ = S // 2          # query rows per core
P = 128
DC = D // P          # 2 chunks of the d/o dimension
MSUP = 512           # m-super: moving free dim per score matmul
NSUP = NQ // MSUP    # 4 m-supers per core
NKC = S // P         # 32 key chunks
GN = 2               # key chunks per group (DoubleRow pair)
NG = NKC // GN       # 16 groups per m-super
NTIL = S // (8 * P)  # 4 k/v tiles (1024 rows each)
SCALE = 1.0 / 16.0   # 1/sqrt(D)
NPE_SUM = 16         # trailing groups per m-super whose rowsum runs on PE
DVE_EXP = (2, 5, 8, 11, 14)  # groups whose exp runs on the DVE
# Scores are pre-scaled by A (folded into the q projection); exp via
# Schraudolph bits rne(A*score + B) on DVE, or Exp(score*A/(8 log2e)) on ACT
EXP_A = 8 * 1.4426950408889634 / 16
EXP_B = 56.0 - 8 * 0.0597
ACT_SCALE = 1.0 / (8 * 1.4426950408889634)

_CACHE = {}
DEBUG_DUMP = False


def _build(num_devices=NCORES):
    import concourse.mybir as mybir
    import concourse.tile as tile
    from concourse import bacc
    from concourse.masks import make_identity

    f32 = mybir.dt.float32
    bf16 = mybir.dt.bfloat16
    fp8 = mybir.dt.float8e4
    i8 = mybir.dt.int8
    FT = mybir.ActivationFunctionType
    ALU = mybir.AluOpType
    DR = mybir.MatmulPerfMode.DoubleRow

    nc = bacc.Bacc("TRN2", target_bir_lowering=False, debug=False,
                   num_devices=num_devices)
    q_ext = nc.dram_tensor("query", [NQ, D], f32, kind="ExternalInput").ap()
    k_ext = nc.dram_tensor("key", [S, D], f32, kind="ExternalInput").ap()
    v_ext = nc.dram_tensor("value", [S, D], f32, kind="ExternalInput").ap()
    w_ext = nc.dram_tensor("Wq", [D, D], f32, kind="ExternalInput").ap()
    b_ext = nc.dram_tensor("bq", [D], f32, kind="ExternalInput").ap()
    o_ext = nc.dram_tensor("out", [NQ, D], f32, kind="ExternalOutput").ap()
    if DEBUG_DUMP:
        dbg_sa = nc.dram_tensor("dbg_sa", [NSUP, P, MSUP], f32,
                                kind="ExternalOutput").ap()
        dbg_sp = nc.dram_tensor("dbg_sp", [NSUP, P, MSUP], f32,
                                kind="ExternalOutput").ap()
        dbg_pt = nc.dram_tensor("dbg_pt", [P, GN, MSUP], f32,
                                kind="ExternalOutput").ap()

    with contextlib.ExitStack() as ctx:
        tc = ctx.enter_context(tile.TileContext(nc))
        singles = ctx.enter_context(tc.tile_pool(name="singles", bufs=1))
        fstage = ctx.enter_context(tc.tile_pool(name="fstage", bufs=2))
        bstage = ctx.enter_context(tc.tile_pool(name="bstage", bufs=3))
        ptpool = ctx.enter_context(tc.tile_pool(name="ptpool", bufs=4))
        utsbp = ctx.enter_context(tc.tile_pool(name="utsbp", bufs=2))
        rsbp = ctx.enter_context(tc.tile_pool(name="rsbp", bufs=2))
        osbp = ctx.enter_context(tc.tile_pool(name="osbp", bufs=3))
        # PSUM budget (16KB/partition): st 2x4KB + ut 4KB + s 2KB + o 2KB.
        # Prologue transposes (tp) and projections (pp) borrow st/o slots so
        # they can interleave with main-loop tiles without deadlock.
        ps_st = ctx.enter_context(tc.tile_pool(name="ps_st", bufs=2, space="PSUM"))
        ps_ut = ctx.enter_context(tc.tile_pool(name="ps_ut", bufs=1, space="PSUM"))
        ps_s = ctx.enter_context(tc.tile_pool(name="ps_s", bufs=1, space="PSUM"))
        ps_o = ctx.enter_context(tc.tile_pool(name="ps_o", bufs=1, space="PSUM"))

        # ---------------- constants ----------------
        ident_f = singles.tile([P, P], f32, tag="identf")
        make_identity(nc, ident_f)
        ident = singles.tile([P, P], bf16, tag="ident")
        nc.vector.tensor_copy(ident, ident_f)
        # ones: rowsum matmul -> s replicated on all partitions
        ones_f8 = singles.tile([P, GN, P], fp8, tag="ones")
        nc.vector.memset(ones_f8, 1.0)
        ones_bf = singles.tile([P, P], bf16, tag="ones_bf")
        nc.vector.memset(ones_bf, 1.0)
        # bq_pc[p, oc] = bq[oc*P + p] (per-partition bias for projT layouts)
        bq_pc = singles.tile([P, DC], f32, tag="bq_pc")
        nc.sync.dma_start(out=bq_pc, in_=b_ext.rearrange("(c p) -> p c", p=P))
        bq_pcA = singles.tile([P, DC], f32, tag="bq_pcA")
        nc.vector.tensor_scalar_mul(bq_pcA, bq_pc, EXP_A)
        # bq_full[p, o] = bq[o] -- broadcast across partitions (DMA step-0)
        bq_full = singles.tile([P, D], f32, tag="bq_full")
        b_bc = dataclasses.replace(
            b_ext, ap=[[0, P]] + [list(c) for c in b_ext.ap])
        nc.sync.dma_start(out=bq_full, in_=b_bc)

        tp_pools = itertools.cycle([(ps_st, "st"), (ps_o, "o")])
        pp_pools = itertools.cycle([(ps_st, "st"), (ps_o, "o")])

        # wqT[p, dc, o] = Wq[o, dc*P + p]  (bf16 PE transpose of bf16 cast)
        wq_f = fstage.tile([P, DC, D], f32, tag="fst", name="wq_f")
        nc.sync.dma_start(out=wq_f, in_=w_ext.rearrange("(c p) d -> p c d", p=P))
        wq_b = bstage.tile([P, DC, D], bf16, tag="bst", name="wq_b")
        nc.vector.tensor_copy(wq_b, wq_f)
        wqT = singles.tile([P, DC, D], bf16, tag="wqT")
        for dcc in range(DC):
            pool, tag = next(tp_pools)
            tp = pool.tile([P, DC, P], bf16, tag=tag, name=f"wqt_{dcc}")
            for oc in range(DC):
                nc.tensor.transpose(tp[:, oc, :],
                                    wq_b[:, oc, dcc * P:(dcc + 1) * P], ident)
            nc.vector.tensor_copy(wqT[:, dcc, :], tp)

        # ------------- load + cast + transpose helpers -------------
        # paired-row load: tile[p, gi, j, d] = src[(c*4+gi)*256 + 2p + j, d]
        def load_paired(ext, c, nm):
            nat = ext.rearrange("(c p j) d -> p c (j d)", p=P, j=2)
            tf = fstage.tile([P, 4, 2, D], f32, tag="fst", name=f"f{nm}_{c}")
            nc.sync.dma_start(
                out=tf.rearrange("p a b d -> p a (b d)"),
                in_=nat[:, c * 4:(c + 1) * 4, :])
            return tf

        def transpose_tile(tf, c, dst, nm):
            # cast bf16 then PE-transpose; chunk index mc = 8c + 2gi + j
            tb = bstage.tile([P, 4, 2, D], bf16, tag="bst", name=f"b{nm}_{c}")
            nc.vector.tensor_copy(tb, tf)
            for dcc in range(DC):
                for h in range(2):
                    pool, tag = next(tp_pools)
                    tp = pool.tile([P, 4, P], bf16, tag=tag,
                                   name=f"t{nm}_{c}_{dcc}_{h}")
                    for i4 in range(4):
                        gi, j = 2 * h + i4 // 2, i4 % 2
                        nc.tensor.transpose(
                            tp[:, i4, :], tb[:, gi, j, dcc * P:(dcc + 1) * P],
                            ident)
                    mc0 = 8 * c + 4 * h
                    nc.vector.tensor_copy(dst[:, dcc, mc0 * P:(mc0 + 4) * P],
                                          tp)

        def project(srcT, dst, nsi, scaled):
            # dst[p, oc, nsi-slab] = (Wq srcT + bq) [* A] in fp8; add on ACT
            nsl = slice(nsi * MSUP, (nsi + 1) * MSUP)
            for oc in range(DC):
                pool, tag = next(pp_pools)
                pp = pool.tile([P, MSUP], f32, tag=tag,
                               name=f"pp_{id(dst)}_{oc}_{nsi}")
                for dcc in range(DC):
                    nc.tensor.matmul(pp, lhsT=wqT[:, dcc, oc * P:(oc + 1) * P],
                                     rhs=srcT[:, dcc, nsl],
                                     start=(dcc == 0), stop=(dcc == DC - 1))
                if scaled:
                    nc.scalar.activation(out=dst[:, oc, nsl], in_=pp,
                                         func=FT.Identity, scale=EXP_A,
                                         bias=bq_pcA[:, oc:oc + 1])
                else:
                    nc.scalar.activation(out=dst[:, oc, nsl], in_=pp,
                                         func=FT.Identity,
                                         bias=bq_pc[:, oc:oc + 1])

        qT = singles.tile([P, DC, NQ], bf16, tag="qT")
        q_pT = singles.tile([P, DC, NQ], fp8, tag="q_pT")
        kT = singles.tile([P, DC, S], bf16, tag="kT")
        k_pT = singles.tile([P, DC, S], fp8, tag="k_pT")
        v_f8 = singles.tile([P, NKC, D], fp8, tag="v_f8")

        # ------------- main-loop emission helpers -------------
        uts, sps, saps, sts = {}, {}, {}, {}
        o_paired = o_ext.rearrange("(c p j) d -> p c j d", p=P, j=2)

        def epilogue(msi):
            ut_ps = uts.pop(msi)
            s_ps = sps.pop(msi)
            recip = rsbp.tile([P, MSUP], f32, tag="recip", name=f"r_{msi}")
            if NPE_SUM < NG:
                # fold the DVE per-partition partial sums into s_ps: one
                # bf16 cast + a 128-contraction ones-matmul per chunk pair
                sa, _ = saps.pop(msi)
                sab = rsbp.tile([P, GN, MSUP], bf16, tag="sab",
                                name=f"sab_{msi}")
                nc.vector.tensor_copy(sab, sa)
                for i in range(GN):
                    nc.tensor.matmul(s_ps, lhsT=ones_bf, rhs=sab[:, i, :],
                                     start=False, stop=(i == GN - 1),
                                     skip_group_check=True)
            nc.vector.reciprocal_approx_fast(recip, s_ps)
            ut_sb = utsbp.tile([P, DC, MSUP], bf16, tag="utsb",
                               name=f"utsb_{msi}")
            for mc in range(MSUP // P):
                msl = slice(mc * P, (mc + 1) * P)
                for dcc in range(DC):
                    nc.vector.tensor_mul(ut_sb[:, dcc, msl],
                                         ut_ps[:, dcc, msl], recip[:, msl])
                o_ps = ps_o.tile([P, D], f32, tag="o", name=f"o_{msi}_{mc}")
                for dcc in range(DC):
                    nc.tensor.matmul(o_ps,
                                     lhsT=ut_sb[:, dcc, msl],
                                     rhs=wqT[:, dcc, :],
                                     start=(dcc == 0), stop=(dcc == DC - 1),
                                     skip_group_check=True)
                o_sb = osbp.tile([P, D], f32, tag="osb", name=f"ob_{msi}_{mc}")
                nc.vector.tensor_add(o_sb, o_ps, bq_full)
                mcg = msi * (MSUP // P) + mc
                nc.sync.dma_start(out=o_paired[:, mcg // 2, mcg % 2, :],
                                  in_=o_sb)

        TT = NSUP * NG

        pts = {}

        def emit_score(t):
            msi, g = divmod(t, NG)
            if g == 0:
                uts[msi] = ps_ut.tile([P, DC, MSUP], f32, tag="ut",
                                      name=f"ut_{msi}")
                sps[msi] = ps_s.tile([P, MSUP], f32, tag="s", name=f"s_{msi}")
                if NPE_SUM < NG:
                    saps[msi] = [
                        rsbp.tile([P, GN, MSUP], f32, tag="sacca",
                                  name=f"sa_{msi}"),
                        rsbp.tile([P, GN, MSUP], f32, tag="saccb",
                                  name=f"sb_{msi}"),
                    ]
            st = ps_st.tile([P, GN, MSUP], f32, tag="st", name=f"st_{t}")
            msl = slice(msi * MSUP, (msi + 1) * MSUP)
            for i in range(GN):
                kc = g * GN + i
                nc.tensor.matmul(st[:, i, :],
                                 lhsT=k_pT[:, :, kc * P:(kc + 1) * P],
                                 rhs=q_pT[:, :, msl],
                                 start=True, stop=True, perf_mode=DR,
                                 skip_group_check=True)
            # exp issued immediately so ACT/DVE get a full group of lead
            pt = ptpool.tile([P, GN, MSUP], fp8, tag="pt", name=f"pt_{t}")
            if g in DVE_EXP:
                nc.vector.tensor_scalar_add(pt.bitcast(i8), st, EXP_B)
            else:
                nc.scalar.activation(out=pt, in_=st, func=FT.Exp,
                                     scale=ACT_SCALE)
            sts[t] = st
            pts[t] = pt

        def consume(t):
            msi, g = divmod(t, NG)
            sts.pop(t)
            pt = pts.pop(t)
            for dcc in range(DC):
                nc.tensor.matmul(uts[msi][:, dcc, :],
                                 lhsT=v_f8[:, GN * g:GN * (g + 1),
                                           dcc * P:(dcc + 1) * P],
                                 rhs=pt,
                                 start=(g == 0), stop=(g == NG - 1),
                                 perf_mode=DR, skip_group_check=True)
            if g >= NG - NPE_SUM:
                # per-group partition reduction on the PE (replicated rows)
                nc.tensor.matmul(sps[msi], lhsT=ones_f8, rhs=pt,
                                 start=(g == NG - NPE_SUM),
                                 stop=(NPE_SUM >= NG and g == NG - 1),
                                 perf_mode=DR, skip_group_check=True)
            elif g == 0:
                nc.vector.tensor_copy(saps[msi][0], pt)
            else:
                # ping-pong accumulate per-partition partial sums on DVE;
                # the epilogue folds the partition reduction into s_ps
                sa, sb = saps[msi]
                nc.vector.tensor_add(sb, sa, pt)
                saps[msi] = [sb, sa]
            if g == NG - 1:
                epilogue(msi)

        tcur = [0]

        def advance(upto):
            while tcur[0] < upto:
                t = tcur[0]
                if t < TT:
                    emit_score(t)
                if t > 0:
                    consume(t - 1)
                tcur[0] += 1

        # ------------- interleaved prologue + main loop -------------
        # q tile 0 -> q_pT slabs for m-supers 0,1
        tf = load_paired(q_ext, 0, "q")
        transpose_tile(tf, 0, qT, "q")
        for nsi in (0, 1):
            project(qT, q_pT, nsi, scaled=True)

        for c in range(NTIL):
            tf = load_paired(k_ext, c, "k")
            vf = load_paired(v_ext, c, "v")
            transpose_tile(tf, c, kT, "k")
            for nsi in (2 * c, 2 * c + 1):
                project(kT, k_pT, nsi, scaled=False)
            nc.vector.tensor_copy(v_f8[:, c * 8:(c + 1) * 8, :],
                                  vf.rearrange("p a b d -> p (a b) d"))
            advance(4 * (c + 1))
            if c == 0:
                # q tile 1 -> slabs for m-supers 2,3 (needed from t=32)
                tf = load_paired(q_ext, 1, "q")
                transpose_tile(tf, 1, qT, "q")
                for nsi in (2, 3):
                    project(qT, q_pT, nsi, scaled=True)

        advance(TT + 1)

    nc.finalize()
    return nc


def _get_nc():
    if "nc" not in _CACHE:
        _CACHE["nc"] = _build()
    return _CACHE["nc"]


def kernel(query, key, value, Wq, bq):
    from concourse.bass_utils import run_bass_kernel_spmd

    nc = _get_nc()
    in_maps = []
    for core in range(NCORES):
        b, h = core // 2, core % 2
        in_maps.append({
            "query": np.ascontiguousarray(query[b, h * NQ:(h + 1) * NQ, :],
                                          dtype=np.float32),
            "key": np.ascontiguousarray(key[b], dtype=np.float32),
            "value": np.ascontiguousarray(value[b], dtype=np.float32),
            "Wq": np.ascontiguousarray(Wq, dtype=np.float32),
            "bq": np.ascontiguousarray(bq, dtype=np.float32),
        })
    res = run_bass_kernel_spmd(nc, in_maps, core_ids=list(range(NCORES)))
    out = np.empty((B, S, D), np.float32)
    for core in range(NCORES):
        b, h = core // 2, core % 2
        out[b, h * NQ:(h + 1) * NQ, :] = res.results[core]["out"]
    return out
